# revision 1
# baseline (speedup 1.0000x reference)
"""Trainium2 Bass kernel for the DUAN conditioned-normalization problem.

Contract: kernel(**inputs) takes FULL inputs (B=8 samples), shards one sample
per NeuronCore (8 cores), runs a single Bass/Tile kernel SPMD, and gathers the
full [8, 512, 8192] output.

Per-sample math (matches the jax reference):
  mu_c/var_c over L per channel; mu_l/var_l over (C,L);
  g = sigmoid(gw2 @ relu(gw1 @ c + gb1) + gb2); g_mix = mean_L(g)
  gamma,beta = mw2 @ relu(mw1 @ mean_L(c) + mb1) + mb2
  mu = g_mix*mu_c + (1-g_mix)*mu_l ; sigma likewise from sqrt(var+eps)
  y = (1+gamma)*(x-mu)/sigma + beta ; keep top-k channels by mean_L |y|.

V5 layout (x/c/out bf16 -> 24 MiB HBM traffic/core vs 40 in V1):
  Phase 1 streams c+x per 1-MiB supertile (x stays resident in SBUF); PE
  runs the gate matmuls and accumulates mw1@c into one PSUM bank (cond-MLP
  layer 1, so there is no separate cond pooling pass); ACT runs sigmoid with
  accumulator g-means; DVE runs bn_stats channel stats and the relus.  All
  weights arrive as two packed DMAs ahead of the stream.  Finalize folds
  everything into per-channel A=(1+gamma)/sigma, B=beta-mu*A.  Phase 2a
  accumulates imp=sum|A*x+B| with ACT Abs+accumulate ops (4096 wide) and DVE
  mult-add + abs-reduce chains; the top-k mask is rank-by-count against an
  exact fp32 broadcast of imp built from PE transpose + ones outer products;
  2c streams (A*mask)*x+(B*mask) to HBM as bf16.
"""

import sys

sys.path.insert(0, "/opt/trn_rl_repo")

import numpy as np

B = 8
C = 512
L = 8192
H = 128
CG = 4           # channel groups of 128 partitions
SL = 1024        # phase-1 supertile width along L
NST = L // SL    # 8
LB = 512         # bn_stats / matmul block
NLB = L // LB    # 16
XC = 2048        # phase-1 slot width along L
NXC = L // XC    # 4
YC = 4096        # fused phase-2 chunk
NYC = L // YC    # 2
KEEP = max(1, int(C * 0.7))  # 358
EPS = 1e-5

# packed f32 weight layout (columns)
F_GB1 = 0
F_GB2 = 1
F_MB1 = 5
F_MB2 = 6
F_M2 = 14
F_ID = F_M2 + 2 * C        # 1038
F_IOTA = F_ID + 128        # 1166
NF = F_IOTA + CG           # 1170

# 2a split: ACT gets (g, j) 4096-wide pairs (j, j+1), DVE 2048 chunks
ACT_2A = ((0, 0), (0, 2), (1, 0), (1, 2), (2, 0))
DVE_2A = ((2, 2), (2, 3), (3, 0), (3, 1), (3, 2), (3, 3))

_CACHE = {}


def _build_nc():
    import concourse.bacc as bacc
    import concourse.bass as bass
    import concourse.tile as tile
    from concourse import mybir

    f32 = mybir.dt.float32
    bf16 = mybir.dt.bfloat16
    i32 = mybir.dt.int32
    AF = mybir.ActivationFunctionType
    OP = mybir.AluOpType
    AX = mybir.AxisListType

    nc = bacc.Bacc("TRN2", target_bir_lowering=False, debug=False, num_devices=8)

    x_d = nc.declare_dram_parameter("x", [C, L], bf16, isOutput=False)
    c_d = nc.declare_dram_parameter("c", [C, L], bf16, isOutput=False)
    wpb_d = nc.declare_dram_parameter("wpk_bf", [128, 1536], bf16, isOutput=False)
    wpf_d = nc.declare_dram_parameter("wpk_f", [128, NF], f32, isOutput=False)
    out_d = nc.declare_dram_parameter("out", [C, L], bf16, isOutput=True)

    with tile.TileContext(nc) as tc:
        _emit(tc, bass, mybir, f32, bf16, i32, AF, OP, AX,
              x_d, c_d, wpb_d, wpf_d, out_d)

    nc.compile()
    return nc


def _emit(tc, bass, mybir, f32, bf16, i32, AF, OP, AX,
          x_d, c_d, wpb_d, wpf_d, out_d):
    from contextlib import ExitStack

    nc = tc.nc

    with ExitStack() as top:
        xpool = top.enter_context(tc.tile_pool(name="xbuf", bufs=1))
        wpool = top.enter_context(tc.tile_pool(name="wts", bufs=1))
        spool = top.enter_context(tc.tile_pool(name="stats", bufs=1))
        ps_m = top.enter_context(tc.tile_pool(name="psm", bufs=1, space="PSUM"))
        m1_ps = ps_m.tile([128, LB], f32, tag="m1ps", name="m1ps")

        # ---- packed weights ----
        wb = wpool.tile([128, 1536], bf16, tag="wb", name="wb")
        nc.sync.dma_start(out=wb[:], in_=wpb_d[:])
        wf = wpool.tile([128, NF], f32, tag="wf", name="wf")
        nc.sync.dma_start(out=wf[:], in_=wpf_d[:])

        def w1g(g):
            return wb[:, g * 128:(g + 1) * 128]

        def m1g(g):
            return wb[:, 512 + g * 128:512 + (g + 1) * 128]

        def w2g(g):
            return wb[:, 1024 + g * 128:1024 + (g + 1) * 128]

        gb1 = wf[:, F_GB1:F_GB1 + 1]
        mb1 = wf[:, F_MB1:F_MB1 + 1]
        ident_sb = wf[:, F_ID:F_ID + 128]
        iota4g = wf[:, F_IOTA:F_IOTA + CG]

        ones_sb = spool.tile([128, 128], f32, tag="ones", name="ones")
        nc.vector.memset(ones_sb[:], 1.0)

        # ---- persistent accumulators / small tiles ----
        X_sb = [xpool.tile([128, L], bf16, tag=f"X{g}", name=f"X{g}") for g in range(CG)]
        stats = [spool.tile([128, NLB, 6], f32, tag=f"bnst{g}", name=f"bnst{g}")
                 for g in range(CG)]
        gacc = spool.tile([128, CG, NST], f32, tag="gacc", name="gacc")
        impacc = spool.tile([128, CG, NXC], f32, tag="impacc", name="impacc")
        nc.vector.memset(impacc[:], 0.0)
        muvar = spool.tile([128, CG, 2], f32, tag="muvar", name="muvar")
        work = spool.tile([128, 16], f32, tag="work", name="work")
        scal = spool.tile([128, 8], f32, tag="scal", name="scal")
        bl_sb = spool.tile([128, 2], f32, tag="blb", name="blb")
        gm4 = spool.tile([128, CG], f32, tag="gm4", name="gm4")
        mu4t = spool.tile([128, CG], f32, tag="mu4t", name="mu4t")
        sg4t = spool.tile([128, CG], f32, tag="sg4t", name="sg4t")
        imp4 = spool.tile([128, CG], f32, tag="imp4", name="imp4")
        A4 = spool.tile([128, CG], f32, tag="A4", name="A4")
        B4 = spool.tile([128, CG], f32, tag="B4", name="B4")
        rank4 = spool.tile([128, CG], f32, tag="rank4", name="rank4")
        mask4 = spool.tile([128, CG], f32, tag="mask4", name="mask4")
        idx4f = spool.tile([128, CG], f32, tag="idx4f", name="idx4f")
        idx4i_f = spool.tile([128, CG], f32, tag="idx4i", name="idx4i_f")
        hm_sb = spool.tile([128, 1], f32, tag="hm", name="hm")
        tr_sb = spool.tile([1, CG, 128], f32, tag="tr4", name="tr4")
        T_sb = spool.tile([128, C], f32, tag="Tsb", name="Tsb")
        G_sb = spool.tile([128, C], f32, tag="Gsb", name="Gsb")

        # =========================== phase 1 ===========================
        with ExitStack() as ph1:
            cpool = ph1.enter_context(tc.tile_pool(name="cbuf", bufs=3))
            hpool = ph1.enter_context(tc.tile_pool(name="hbuf", bufs=2))
            gspool = ph1.enter_context(tc.tile_pool(name="gscr", bufs=4))
            ps_h = ph1.enter_context(tc.tile_pool(name="psh", bufs=1, space="PSUM"))
            ps_g = ph1.enter_context(tc.tile_pool(name="psg", bufs=2, space="PSUM"))

            for k in range(NXC):
                l0 = k * XC
                c_t = cpool.tile([128, CG, XC], bf16, tag="ct", name="ct")
                cap = c_d[:]
                for half in range(2):
                    c_src = bass.AP(tensor=cap.tensor, offset=l0 + half * SL,
                                    ap=[[L, 128], [128 * L, CG], [1, SL]])
                    nc.sync.dma_start(out=c_t[:, :, half * SL:(half + 1) * SL],
                                      in_=c_src)
                    for g in range(2):
                        gg = half * 2 + g
                        nc.sync.dma_start(
                            out=X_sb[gg][:, l0:l0 + XC],
                            in_=x_d[gg * 128:(gg + 1) * 128, l0:l0 + XC])

                # DVE order: keep each slot's relus near the queue front so
                # the ACT sigmoid stream starts early; slot 0 interleaves the
                # early-arriving group-0/1 stats ahead of the gate warm-up.
                def emit_bn(gs):
                    for g in gs:
                        for hh in range(NLB // NXC):
                            j = k * (NLB // NXC) + hh
                            nc.vector.bn_stats(out=stats[g][:, j, :],
                                               in_=X_sb[g][:, j * LB:(j + 1) * LB])

                if k == 0:
                    emit_bn((0, 1))

                for half in range(2):
                    st = k * 2 + half
                    # gate layer 1: h = relu(gw1 @ c + gb1); relu on DVE
                    h_ps = ps_h.tile([128, SL], f32, tag="hps", name="hps")
                    for g in range(CG):
                        for hh in range(2):
                            hs = slice(hh * LB, (hh + 1) * LB)
                            ds = slice(half * SL + hh * LB,
                                       half * SL + (hh + 1) * LB)
                            nc.tensor.matmul(h_ps[:, hs], w1g(g),
                                             c_t[:, g, ds],
                                             start=(g == 0), stop=(g == CG - 1))
                    h_sb = hpool.tile([128, SL], bf16, tag="hsb", name="hsb")
                    nc.vector.tensor_scalar(out=h_sb[:], in0=h_ps[:],
                                            scalar1=gb1,
                                            scalar2=0.0,
                                            op0=OP.add, op1=OP.max)

                    # cond-MLP layer 1: accumulate mw1 @ c into one PSUM bank
                    for g in range(CG):
                        for hh in range(2):
                            ds = slice(half * SL + hh * LB,
                                       half * SL + (hh + 1) * LB)
                            first = (k == 0 and half == 0 and g == 0 and hh == 0)
                            last = (k == NXC - 1 and half == 1
                                    and g == CG - 1 and hh == 1)
                            nc.tensor.matmul(m1_ps[:], m1g(g),
                                             c_t[:, g, ds],
                                             start=first, stop=last)

                    # gate layer 2 + sigmoid (+ g_mix accumulator on ACT)
                    for g in range(CG):
                        g_ps = ps_g.tile([128, SL], f32, tag="gps", name="g_ps")
                        for hh in range(2):
                            hs = slice(hh * LB, (hh + 1) * LB)
                            nc.tensor.matmul(g_ps[:, hs], w2g(g),
                                             h_sb[:, hs], start=True, stop=True)
                        g_scr = gspool.tile([128, SL], bf16, tag="gscr", name="gscr")
                        nc.scalar.activation(out=g_scr[:], in_=g_ps[:],
                                             func=AF.Sigmoid,
                                             bias=wf[:, F_GB2 + g:F_GB2 + g + 1],
                                             scale=1.0,
                                             accum_out=gacc[:, g, st:st + 1])

                emit_bn((2, 3) if k == 0 else (0, 1, 2, 3))

        # =========================== finalize ===========================
        with ExitStack() as fin:
            ps_f = fin.enter_context(tc.tile_pool(name="psf", bufs=1, space="PSUM"))

            for g in range(CG):
                nc.vector.bn_aggr(out=muvar[:, g, :], in_=stats[g][:])
            mu_c = work[:, 0:4]
            var_c = muvar[:, :, 1]
            nc.vector.tensor_copy(out=mu_c, in_=muvar[:, :, 0])
            # work 4:8 = E[x^2] = var_c + mu_c^2
            nc.vector.tensor_tensor(out=work[:, 4:8], in0=mu_c, in1=mu_c,
                                    op=OP.mult)
            nc.vector.tensor_add(out=work[:, 4:8], in0=work[:, 4:8], in1=var_c)

            # cross-partition sums via ones-matmul -> [1, 8]
            colsum = ps_f.tile([128, 8], f32, tag="colsum", name="colsum")
            nc.tensor.matmul(colsum[0:1, :], ones_sb[:, 0:1], work[:, 0:8],
                             start=True, stop=True)

            # partition-0 scalars: mu_l, sigma_l
            nc.vector.reduce_sum(out=scal[0:1, 0:1], in_=colsum[0:1, 0:4], axis=AX.X)
            nc.vector.tensor_scalar(out=scal[0:1, 0:1], in0=scal[0:1, 0:1],
                                    scalar1=1.0 / C, scalar2=None, op0=OP.mult)
            nc.vector.reduce_sum(out=scal[0:1, 2:3], in_=colsum[0:1, 4:8], axis=AX.X)
            nc.vector.tensor_scalar(out=scal[0:1, 2:3], in0=scal[0:1, 2:3],
                                    scalar1=1.0 / C, scalar2=None, op0=OP.mult)
            nc.vector.tensor_tensor(out=scal[0:1, 3:4], in0=scal[0:1, 0:1],
                                    in1=scal[0:1, 0:1], op=OP.mult)
            nc.vector.tensor_tensor(out=scal[0:1, 1:2], in0=scal[0:1, 2:3],
                                    in1=scal[0:1, 3:4], op=OP.subtract)
            nc.vector.tensor_scalar(out=scal[0:1, 1:2], in0=scal[0:1, 1:2],
                                    scalar1=EPS, scalar2=None, op0=OP.add)
            nc.scalar.activation(out=scal[0:1, 1:2], in_=scal[0:1, 1:2],
                                 func=AF.Sqrt, bias=0.0, scale=1.0)

            # broadcast (mu_l, sigma_l) to all partitions
            bl_ps = ps_f.tile([128, 2], f32, tag="blps", name="blps")
            nc.tensor.matmul(bl_ps[:], ones_sb[0:1, :], scal[0:1, 0:2],
                             start=True, stop=True)
            nc.vector.tensor_copy(out=bl_sb[:], in_=bl_ps[:])
            mu_l = bl_sb[:, 0:1]
            sig_l = bl_sb[:, 1:2]

            # sigma_c = sqrt(var_c + eps)
            vpe4 = work[:, 8:12]
            sig4 = work[:, 12:16]
            nc.vector.tensor_scalar(out=vpe4, in0=var_c, scalar1=EPS,
                                    scalar2=None, op0=OP.add)
            nc.scalar.activation(out=sig4, in_=vpe4, func=AF.Sqrt,
                                 bias=0.0, scale=1.0)

            # g_mix
            nc.vector.tensor_reduce(out=gm4[:], in_=gacc[:], axis=AX.X, op=OP.add)
            nc.vector.tensor_scalar(out=gm4[:], in0=gm4[:], scalar1=1.0 / L,
                                    scalar2=None, op0=OP.mult)

            # mu = mu_l + g_mix*(mu_c - mu_l); sigma likewise
            nc.vector.tensor_scalar(out=mu4t[:], in0=mu_c, scalar1=mu_l,
                                    scalar2=None, op0=OP.subtract)
            nc.vector.tensor_tensor(out=mu4t[:], in0=mu4t[:], in1=gm4[:], op=OP.mult)
            nc.vector.tensor_scalar(out=mu4t[:], in0=mu4t[:], scalar1=mu_l,
                                    scalar2=None, op0=OP.add)
            nc.vector.tensor_scalar(out=sg4t[:], in0=sig4, scalar1=sig_l,
                                    scalar2=None, op0=OP.subtract)
            nc.vector.tensor_tensor(out=sg4t[:], in0=sg4t[:], in1=gm4[:], op=OP.mult)
            nc.vector.tensor_scalar(out=sg4t[:], in0=sg4t[:], scalar1=sig_l,
                                    scalar2=None, op0=OP.add)

            # cond MLP: hm = relu(mean_L(mw1 @ c) + mb1)
            nc.vector.reduce_sum(out=hm_sb[:], in_=m1_ps[:], axis=AX.X)
            nc.scalar.activation(out=hm_sb[:], in_=hm_sb[:], func=AF.Relu,
                                 bias=mb1, scale=1.0 / L)
            gb_ps = ps_f.tile([128, 2 * CG], f32, tag="gbps", name="gbps")
            for o in range(2 * CG):
                nc.tensor.matmul(gb_ps[:, o:o + 1],
                                 wf[:, F_M2 + o * 128:F_M2 + (o + 1) * 128],
                                 hm_sb[:], start=True, stop=True)

            # A = (1+gamma)/sigma ; B = beta - mu*A
            inv4 = work[:, 8:12]
            nc.vector.reciprocal(out=inv4, in_=sg4t[:])
            gam4 = work[:, 12:16]
            nc.vector.tensor_add(out=gam4, in0=gb_ps[:, 0:CG],
                                 in1=wf[:, F_MB2:F_MB2 + CG])
            nc.vector.tensor_scalar(out=gam4, in0=gam4, scalar1=1.0,
                                    scalar2=None, op0=OP.add)
            bet4 = work[:, 4:8]
            nc.vector.tensor_add(out=bet4, in0=gb_ps[:, CG:2 * CG],
                                 in1=wf[:, F_MB2 + CG:F_MB2 + 2 * CG])
            muA = work[:, 0:4]
            nc.vector.tensor_tensor(out=A4[:], in0=gam4, in1=inv4, op=OP.mult)
            nc.vector.tensor_tensor(out=muA, in0=mu4t[:], in1=A4[:], op=OP.mult)
            nc.vector.tensor_tensor(out=B4[:], in0=bet4, in1=muA, op=OP.subtract)

        # =========================== phase 2 ===========================
        with ExitStack() as ph2:
            ypool = ph2.enter_context(tc.tile_pool(name="ybuf", bufs=4))
            apool = ph2.enter_context(tc.tile_pool(name="abuf", bufs=2))
            ps_t = ph2.enter_context(tc.tile_pool(name="pst", bufs=1, space="PSUM"))

            # 2a: imp = sum_L |A*x + B| ; ACT 4096-pairs, DVE 2048 chunks
            alt = []
            for i in range(max(len(ACT_2A), len(DVE_2A))):
                if i < len(ACT_2A):
                    alt.append(("act",) + ACT_2A[i])
                if i < len(DVE_2A):
                    alt.append(("dve",) + DVE_2A[i])
            for eng, g, j in alt:
                if eng == "act":
                    xa = X_sb[g][:, j * XC:(j + 2) * XC]
                    scr = apool.tile([128, 2 * XC], bf16, tag="ascr", name="ascr")
                    nc.scalar.activation(out=scr[:], in_=xa, func=AF.Abs,
                                         bias=B4[:, g:g + 1],
                                         scale=A4[:, g:g + 1],
                                         accum_out=impacc[:, g, j:j + 1])
                else:
                    xa = X_sb[g][:, j * XC:(j + 1) * XC]
                    y_scr = apool.tile([128, XC], bf16, tag="yscr", name="y_scr")
                    nc.vector.tensor_scalar(out=y_scr[:], in0=xa,
                                            scalar1=A4[:, g:g + 1],
                                            scalar2=B4[:, g:g + 1],
                                            op0=OP.mult, op1=OP.add)
                    nc.vector.tensor_reduce(out=impacc[:, g, j:j + 1],
                                            in_=y_scr[:], axis=AX.X,
                                            op=OP.add, apply_absolute_value=True)
            nc.vector.tensor_reduce(out=imp4[:], in_=impacc[:], axis=AX.X, op=OP.add)

            # 2b: exact fp32 broadcast of imp via PE transpose + ones outer
            tr_ps = ps_t.tile([1, CG, 128], f32, tag="trps", name="trps")
            for g in range(CG):
                nc.tensor.matmul(tr_ps[0:1, g, :], imp4[:, g:g + 1],
                                 ident_sb, is_transpose=True,
                                 start=True, stop=True)
            nc.vector.tensor_copy(out=tr_sb[:], in_=tr_ps[:])
            T_ps = ps_t.tile([128, C], f32, tag="Tps", name="Tps")
            for g in range(CG):
                nc.tensor.matmul(T_ps[:, g * 128:(g + 1) * 128],
                                 ones_sb[0:1, 0:128], tr_sb[0:1, g, :],
                                 start=True, stop=True)
            nc.vector.tensor_copy(out=T_sb[:], in_=T_ps[:])
            for g in range(CG):
                nc.vector.tensor_scalar(out=G_sb[:], in0=T_sb[:],
                                        scalar1=imp4[:, g:g + 1], scalar2=0.0,
                                        op0=OP.is_gt, op1=OP.add,
                                        accum_out=rank4[:, g:g + 1])
            nc.vector.tensor_scalar(out=mask4[:], in0=rank4[:], scalar1=float(KEEP),
                                    scalar2=None, op0=OP.is_lt)
            nc.vector.tensor_tensor(out=idx4f[:], in0=A4[:], in1=mask4[:],
                                    op=OP.mult)
            nc.vector.tensor_tensor(out=idx4i_f[:], in0=B4[:], in1=mask4[:],
                                    op=OP.mult)

            # 2c: out = (A*mask)*x + (B*mask) -> bf16 -> HBM
            for idx in range(CG * NXC):
                g, j = divmod(idx, NXC)
                y_t = ypool.tile([128, XC], bf16, tag="yt", name="yt")
                xa = X_sb[g][:, j * XC:(j + 1) * XC]
                nc.vector.tensor_scalar(out=y_t[:], in0=xa,
                                        scalar1=idx4f[:, g:g + 1],
                                        scalar2=idx4i_f[:, g:g + 1],
                                        op0=OP.mult, op1=OP.add)
                nc.sync.dma_start(
                    out=out_d[g * 128:(g + 1) * 128, j * XC:(j + 1) * XC],
                    in_=y_t[:])


def _get_nc():
    if "nc" not in _CACHE:
        _CACHE["nc"] = _build_nc()
    return _CACHE["nc"]


def _host_weight_maps(gw1, gb1, gw2, gb2, mw1, mb1, mw2, mb2):
    import ml_dtypes
    f = np.float32
    bf = ml_dtypes.bfloat16
    w1t = np.asarray(gw1, f).T.reshape(CG, 128, H).transpose(1, 0, 2)  # [128,CG,H]
    m1t = np.asarray(mw1, f).T.reshape(CG, 128, H).transpose(1, 0, 2)
    w2t = np.asarray(gw2, f).T                                          # [H,C]
    wpk_bf = np.concatenate(
        [w1t.reshape(128, 512), m1t.reshape(128, 512), w2t], axis=1).astype(bf)
    wpk_f = np.zeros((128, NF), f)
    wpk_f[:, F_GB1] = np.asarray(gb1, f)
    wpk_f[:, F_GB2:F_GB2 + CG] = np.asarray(gb2, f).reshape(CG, 128).T
    wpk_f[:, F_MB1] = np.asarray(mb1, f)
    wpk_f[:, F_MB2:F_MB2 + 2 * CG] = np.asarray(mb2, f).reshape(2 * CG, 128).T
    wpk_f[:, F_M2:F_M2 + 2 * C] = np.asarray(mw2, f).T                  # [H,2C]
    wpk_f[:, F_ID:F_ID + 128] = np.eye(128, dtype=f)
    iota = np.arange(128, dtype=f)
    for g in range(CG):
        wpk_f[:, F_IOTA + g] = g * 128 + iota
    return {
        "wpk_bf": np.ascontiguousarray(wpk_bf),
        "wpk_f": np.ascontiguousarray(wpk_f),
    }


def _run(inputs, trace=False):
    import ml_dtypes
    from concourse.bass_utils import run_bass_kernel_spmd

    nc = _get_nc()
    bf = ml_dtypes.bfloat16
    x = np.asarray(inputs["x"], np.float32).astype(bf)
    c = np.asarray(inputs["c"], np.float32).astype(bf)
    wmap = _host_weight_maps(
        inputs["gw1"], inputs["gb1"], inputs["gw2"], inputs["gb2"],
        inputs["mw1"], inputs["mb1"], inputs["mw2"], inputs["mb2"])
    in_maps = [
        dict(wmap, x=np.ascontiguousarray(x[b]), c=np.ascontiguousarray(c[b]))
        for b in range(B)
    ]
    res = run_bass_kernel_spmd(nc, in_maps, core_ids=list(range(B)), trace=trace)
    out = np.stack([np.asarray(res.results[b]["out"], np.float32) for b in range(B)],
                   axis=0)
    return out, res


def kernel(**inputs):
    out, _ = _run(inputs, trace=False)
    return out

